# revision 1
# baseline (speedup 1.0000x reference)
"""Trainium2 Bass kernel: 3-layer mean-aggregation SAGE GNN message passing.

Strategy (8 NeuronCores, SPMD single NEFF):
  - Nodes sharded contiguously: core c owns rows [c*NSH, (c+1)*NSH).
  - Input projection h0 = tanh(x @ W_in + b_in) computed from a CPU-pretransposed
    feature-major x shard (xT), so no on-device transpose is needed for it.
  - Per layer:
      * halo exchange: each core dma_gather's the deduplicated set of its h rows
        that each peer's edges reference (send lists precomputed on CPU from
        edge_index) into an AllToAll bounce buffer; one AllToAll delivers every
        core the boundary rows it needs (own rows are routed through the self
        slot so the SPMD program is uniform across cores).
      * aggregation: edges (dst-sharded, bucketed by src core) are processed as
        dma_gather (from the A2A output slot of the src core) followed by
        dma_scatter_add into a local [NSHP+128, D] accumulator table in HBM.
        Padding edge slots scatter into the dump rows beyond NSHP.
      * node update: per 128-node tile: scale agg by 1/deg (per-partition
        scalar), PE-transpose h and agg tiles, three accumulating matmuls
        (h@W_self + agg@W_neigh + 1x128 bias rank-1 update), activation, and a
        slab DMA back to the next h table (or the output tensor).
All index/bucket preprocessing is pure edge_index/shape metadata computed on
CPU in numpy; all h-dependent compute runs on the NeuronCores.
"""

import math
import os
import sys
from contextlib import ExitStack

import numpy as np

if "/opt/trn_rl_repo" not in sys.path:
    sys.path.insert(0, "/opt/trn_rl_repo")

import concourse.bacc as bacc
import concourse.bass as bass
import concourse.mybir as mybir
import concourse.tile as tile
from concourse.bass_utils import run_bass_kernel_spmd
from concourse.library_config import mlp as _mlp_lib
from concourse.masks import make_identity

P = 128
D = 128
CHUNK = 1024  # edges per dma_gather/dma_scatter_add call (SWDGE ring limit)
F32 = mybir.dt.float32
I16 = mybir.dt.int16


def _roundup(a, m):
    return (a + m - 1) // m * m


# dma_scatter_add's CCE add reads the destination stream ahead of its writes,
# so duplicate dst rows WITHIN one call lose updates. Calls are serialized by
# Tile's WAW ordering, so accumulation across calls is safe. We therefore
# split each bucket into: section A = first occurrence of each dst (unique),
# and section B = repeat occurrences, which are combined on-chip into one
# "leader" row per dst (selection matmul) before their scatter call.


def _bucket_sections(dst_local):
    """Returns (idxA, B_groups) where idxA indexes the unique-dst first
    occurrences and B_groups is a list of per-128-slot groups, each a list of
    (edge_idx_array_for_one_dst_run)."""
    n = dst_local.shape[0]
    order = np.argsort(dst_local, kind="stable")
    ds = dst_local[order]
    first = np.ones(n, dtype=bool)
    first[1:] = ds[1:] != ds[:-1]
    run_id = np.cumsum(first) - 1
    run_start = np.flatnonzero(first)
    occ = np.arange(n) - run_start[run_id]
    idxA = order[occ == 0]
    idxB = order[occ >= 1]  # dst-grouped order preserved
    runsB = []
    if len(idxB):
        dB = dst_local[idxB]
        firstB = np.ones(len(idxB), dtype=bool)
        firstB[1:] = dB[1:] != dB[:-1]
        startsB = np.flatnonzero(firstB).tolist() + [len(idxB)]
        runsB = [idxB[startsB[k] : startsB[k + 1]] for k in range(len(startsB) - 1)]
    # pack runs into 128-slot groups without splitting a run
    groups = []
    cur, cur_n = [], 0
    for r in runsB:
        if cur_n + len(r) > P:
            groups.append(cur)
            cur, cur_n = [], 0
        cur.append(r)
        cur_n += len(r)
    if cur:
        groups.append(cur)
    return idxA, groups


def _wrap16(idx, pad_to, pad_val):
    """[n] int array -> [128, pad_to//16] int16 in the SWDGE wrapped layout:
    element i lives at [i % 16, i // 16], replicated 8x down partitions."""
    n = idx.shape[0]
    full = np.full(pad_to, pad_val, dtype=np.int64)
    full[:n] = idx
    w = full.reshape(pad_to // 16, 16).T.astype(np.int16)
    return np.ascontiguousarray(np.tile(w, (8, 1)))


def preprocess(x, edge_index, W_in, b_in, W_self, W_neigh, b_layers, C):
    """CPU-side structural preprocessing. Returns (meta, in_maps)."""
    x = np.asarray(x, dtype=np.float32)
    src = np.asarray(edge_index[0], dtype=np.int64)
    dst = np.asarray(edge_index[1], dtype=np.int64)
    W_in = np.asarray(W_in, dtype=np.float32)
    b_in = np.asarray(b_in, dtype=np.float32)
    W_self = np.asarray(W_self, dtype=np.float32)
    W_neigh = np.asarray(W_neigh, dtype=np.float32)
    b_layers = np.asarray(b_layers, dtype=np.float32)

    N, F = x.shape
    L = W_self.shape[0]
    assert N % C == 0
    NSH = N // C
    NSHP = _roundup(NSH, P)
    TILES = NSHP // P

    deg = np.bincount(dst, minlength=N).astype(np.float32)
    invd = (1.0 / np.maximum(deg, 1.0)).astype(np.float32)

    dcore = dst // NSH
    score = src // NSH

    order = np.lexsort((score, dcore))
    src_o = src[order]
    dst_o = dst[order]
    sc_o = score[order]
    dc_o = dcore[order]

    counts = np.zeros((C, C), dtype=np.int64)
    np.add.at(counts, (dc_o, sc_o), 1)

    # slice offsets of each (dst_core, src_core) bucket in the sorted edge list
    flat_counts = counts.reshape(-1)
    starts = np.zeros(C * C + 1, dtype=np.int64)
    np.cumsum(flat_counts, out=starts[1:])

    def bucket_slice(dc, sc):
        k = dc * C + sc
        return slice(int(starts[k]), int(starts[k + 1]))

    # send_lists[i][j]: sorted unique src rows of core i's shard referenced by
    # core j's edges (includes i == j so the SPMD program is uniform).
    send_lists = [[None] * C for _ in range(C)]
    sections = {}
    SP = P
    APAD = P
    BGROUPS = 0
    for j in range(C):
        for i in range(C):
            sl = bucket_slice(j, i)
            u = np.unique(src_o[sl])
            send_lists[i][j] = u
            SP = max(SP, len(u))
            idxA, groups = _bucket_sections(dst_o[sl] - j * NSH)
            sections[(j, i)] = (idxA, groups)
            APAD = max(APAD, len(idxA))
            BGROUPS = max(BGROUPS, len(groups))
    SP = int(_roundup(SP, P))
    APAD = int(_roundup(APAD, P))
    BPAD = BGROUPS * P
    BE = APAD + BPAD

    meta = dict(
        C=C, N=N, F=F, L=L, NSH=NSH, NSHP=NSHP, TILES=TILES, BE=BE, SP=SP,
        APAD=APAD, BGROUPS=BGROUPS,
    )

    # weights shared by all cores
    W_in_pad = np.zeros((P, D), dtype=np.float32)
    W_in_pad[:F] = W_in
    b_all = np.concatenate([b_in[None, :], b_layers], axis=0).astype(np.float32)

    in_maps = []
    for c in range(C):
        g_idx = np.empty((C, P, BE // 16), dtype=np.int16)
        s_idx = np.empty((C, P, BE // 16), dtype=np.int16)
        sg_idx = np.empty((C, P, SP // 16), dtype=np.int16)
        cmb_a = np.full((C, P, max(BGROUPS, 1)), -1.0, dtype=np.float32)
        cmb_b = np.full((C, P, max(BGROUPS, 1)), -1.0, dtype=np.float32)
        for b in range(C):
            sl = bucket_slice(c, b)
            sl_src = src_o[sl]
            gi = np.searchsorted(send_lists[b][c], sl_src)
            si = dst_o[sl] - c * NSH
            idxA, groups = sections[(c, b)]
            g_full = np.zeros(BE, dtype=np.int64)
            # pad/non-leader slots scatter into the dump rows beyond NSHP
            s_full = NSHP + (np.arange(BE) % P)
            g_full[: len(idxA)] = gi[idxA]
            s_full[: len(idxA)] = si[idxA]
            for g, runs in enumerate(groups):
                pos = APAD + g * P
                for r in runs:
                    np_r = len(r)
                    g_full[pos : pos + np_r] = gi[r]
                    s_full[pos] = si[r[0]]  # leader carries the dst
                    cmb_a[b, (pos % P) : (pos % P) + np_r, g] = si[r]
                    cmb_b[b, pos % P, g] = si[r[0]]
                    pos += np_r
            g_idx[b] = _wrap16(g_full, BE, 0)
            s_idx[b] = _wrap16(s_full, BE, 0)
            snd = send_lists[c][b] - c * NSH
            sg_idx[b] = _wrap16(snd, SP, 0)

        invd_sh = np.ones(NSHP, dtype=np.float32)
        invd_sh[:NSH] = invd[c * NSH : (c + 1) * NSH]
        invd_t = np.ascontiguousarray(invd_sh.reshape(TILES, P).T)

        xT = np.zeros((P, NSHP), dtype=np.float32)
        xT[:F, :NSH] = x[c * NSH : (c + 1) * NSH].T

        in_maps.append(
            dict(
                xT=xT,
                g_idx=g_idx,
                s_idx=s_idx,
                sg_idx=sg_idx,
                cmb_a=cmb_a,
                cmb_b=cmb_b,
                invd=invd_t,
                w_in=W_in_pad,
                w_self=W_self,
                w_neigh=W_neigh,
                b_all=b_all,
            )
        )
    return meta, in_maps


def _edge_chunks(total):
    out = []
    o = 0
    while o < total:
        n = min(CHUNK, total - o)
        out.append((o, n))
        o += n
    return out


def build_nc(meta):
    C = meta["C"]
    L = meta["L"]
    NSHP = meta["NSHP"]
    TILES = meta["TILES"]
    BE = meta["BE"]
    SP = meta["SP"]
    APAD = meta["APAD"]
    BGROUPS = meta["BGROUPS"]
    BPAD = BGROUPS * P
    AGG_ROWS = NSHP + P
    SLAB = 4 if TILES % 4 == 0 else 1  # node tiles per DMA slab

    nc = bacc.Bacc(
        "TRN2",
        target_bir_lowering=False,
        debug=False,
        num_devices=C,
    )

    xT_t = nc.dram_tensor("xT", [P, NSHP], F32, kind="ExternalInput")
    g_idx_t = nc.dram_tensor("g_idx", [C, P, BE // 16], I16, kind="ExternalInput")
    s_idx_t = nc.dram_tensor("s_idx", [C, P, BE // 16], I16, kind="ExternalInput")
    sg_idx_t = nc.dram_tensor("sg_idx", [C, P, SP // 16], I16, kind="ExternalInput")
    cmb_a_t = nc.dram_tensor(
        "cmb_a", [C, P, max(BGROUPS, 1)], F32, kind="ExternalInput"
    )
    cmb_b_t = nc.dram_tensor(
        "cmb_b", [C, P, max(BGROUPS, 1)], F32, kind="ExternalInput"
    )
    invd_t = nc.dram_tensor("invd", [P, TILES], F32, kind="ExternalInput")
    w_in_t = nc.dram_tensor("w_in", [P, D], F32, kind="ExternalInput")
    w_self_t = nc.dram_tensor("w_self", [L, D, D], F32, kind="ExternalInput")
    w_neigh_t = nc.dram_tensor("w_neigh", [L, D, D], F32, kind="ExternalInput")
    b_all_t = nc.dram_tensor("b_all", [L + 1, D], F32, kind="ExternalInput")
    out_t = nc.dram_tensor("out", [NSHP, D], F32, kind="ExternalOutput")

    AF = mybir.ActivationFunctionType

    with tile.TileContext(nc) as tc, ExitStack() as ctx:
        dram = ctx.enter_context(tc.tile_pool(name="dram", bufs=1, space="DRAM"))
        h_a = dram.tile([NSHP, D], F32, tag="h_a")
        h_b = dram.tile([NSHP, D], F32, tag="h_b")
        agg = dram.tile([AGG_ROWS, D], F32, tag="agg")
        a2a_in = dram.tile([C, SP, D], F32, tag="a2a_in")
        a2a_out = dram.tile([C, SP, D], F32, tag="a2a_out")

        const = ctx.enter_context(tc.tile_pool(name="const", bufs=1))
        sb_idx = ctx.enter_context(tc.tile_pool(name="sb_idx", bufs=4))
        sb_msg = ctx.enter_context(tc.tile_pool(name="sb_msg", bufs=3))
        sb_send = ctx.enter_context(tc.tile_pool(name="sb_send", bufs=2))
        sb_slab = ctx.enter_context(tc.tile_pool(name="sb_slab", bufs=3))
        sb_tr = ctx.enter_context(tc.tile_pool(name="sb_tr", bufs=4))
        sb_out = ctx.enter_context(tc.tile_pool(name="sb_out", bufs=3))
        ps_tr = ctx.enter_context(tc.tile_pool(name="ps_tr", bufs=4, space="PSUM"))
        ps_o = ctx.enter_context(tc.tile_pool(name="ps_o", bufs=3, space="PSUM"))

        nc.gpsimd.load_library(_mlp_lib)


        # --- constants resident in SBUF ---
        ident = const.tile([P, P], F32, tag="ident")
        make_identity(nc, ident[:])
        ones_row = const.tile([1, P], F32, tag="ones_row")
        nc.gpsimd.memset(ones_row[:], 1.0)
        zero_big = const.tile([P, 4096], F32, tag="zero_big")
        nc.gpsimd.memset(zero_big[:], 0.0)
        w_in_sb = const.tile([P, D], F32, tag="w_in_sb")
        nc.sync.dma_start(w_in_sb[:], w_in_t[:, :])
        wself_sb = []
        wneigh_sb = []
        for layer in range(L):
            ws = const.tile([P, D], F32, tag=f"wself{layer}")
            nc.sync.dma_start(ws[:], w_self_t[layer])
            wself_sb.append(ws)
            wn = const.tile([P, D], F32, tag=f"wneigh{layer}")
            nc.sync.dma_start(wn[:], w_neigh_t[layer])
            wneigh_sb.append(wn)
        b_sb = []
        for bi in range(L + 1):
            bt = const.tile([1, D], F32, tag=f"b_sb{bi}")
            nc.sync.dma_start(bt[:], b_all_t[bi : bi + 1, :])
            b_sb.append(bt)
        invd_sb = const.tile([P, TILES], F32, tag="invd_sb")
        nc.sync.dma_start(invd_sb[:], invd_t[:, :])

        def node_update(get_lhs_tile, bias_row, act_fn, h_dst, scale_agg):
            """Per-tile: (transposed) inputs -> matmuls -> act -> slab DMA out.
            get_lhs_tile(s, q, t) returns list of (lhsT_ap, rhs_ap) matmul pairs."""
            for s in range(TILES // SLAB):
                r0 = s * SLAB * P
                rows = SLAB * P
                hn_slab = sb_out.tile([P, SLAB * D], F32, tag="hn_slab")
                pairs_per_q = []
                for q in range(SLAB):
                    t = s * SLAB + q
                    pairs_per_q.append(get_lhs_tile(s, q, t))
                for q in range(SLAB):
                    t = s * SLAB + q
                    po = ps_o.tile([P, D], F32)
                    pairs = pairs_per_q[q]
                    for k, (lhsT, rhs) in enumerate(pairs):
                        nc.tensor.matmul(
                            po[:], lhsT, rhs, start=(k == 0), stop=False
                        )
                    nc.tensor.matmul(
                        po[:], ones_row[:1, :], bias_row, start=False, stop=True
                    )
                    nc.scalar.activation(
                        hn_slab[:, q * D : (q + 1) * D], po[:], act_fn
                    )
                dst_ap = h_dst[r0 : r0 + rows, :].rearrange(
                    "(q p) d -> p q d", p=P
                )
                nc.sync.dma_start(
                    dst_ap, hn_slab[:].rearrange("p (q d) -> p q d", d=D)
                )

        # --- input projection: h0 = tanh(x @ W_in + b_in) -> h_a ---
        def proj_pairs_factory():
            slab_cache = {}

            def get(s, q, t):
                if s not in slab_cache:
                    xsl = sb_slab.tile([P, SLAB * P], F32, tag="x_slab")
                    nc.sync.dma_start(
                        xsl[:], xT_t[:, s * SLAB * P : (s + 1) * SLAB * P]
                    )
                    slab_cache.clear()
                    slab_cache[s] = xsl
                xsl = slab_cache[s]
                return [(xsl[:, q * P : (q + 1) * P], w_in_sb[:])]

            return get

        node_update(proj_pairs_factory(), b_sb[0][:], AF.Tanh, h_a, False)

        h_tabs = [h_a, h_b]

        for layer in range(L):
            h_cur = h_tabs[layer % 2]
            last = layer == L - 1
            h_nxt = out_t if last else h_tabs[(layer + 1) % 2]

            # --- build A2A send buffer: slot j = own h rows peer j needs ---
            for j in range(C):
                for o, n in _edge_chunks(SP):
                    sgi = sb_idx.tile([P, SP // 16], I16, tag="sgi")
                    nc.sync.dma_start(
                        sgi[:, : n // 16],
                        sg_idx_t[j][:, o // 16 : (o + n) // 16],
                    )
                    st = sb_send.tile([P, (CHUNK // P) * D], F32, tag="st")
                    stv = st[:, : (n // P) * D].rearrange(
                        "p (q d) -> p q d", d=D
                    )
                    nc.gpsimd.dma_gather(
                        stv, h_cur[:, :], sgi[:, : n // 16], n, n, D
                    )
                    dst = a2a_in[j][o : o + n, :].rearrange(
                        "(q p) d -> p q d", p=P
                    )
                    nc.sync.dma_start(dst, stv)

            nc.gpsimd.collective_compute(
                "AllToAll",
                mybir.AluOpType.bypass,
                replica_groups=[list(range(C))],
                ins=[a2a_in.opt()],
                outs=[a2a_out.opt()],
            )

            # --- zero the aggregation table ---
            zo = 0
            while zo < AGG_ROWS:
                zn = min(4096, AGG_ROWS - zo)
                dst = agg[zo : zo + zn, :].rearrange("(p r) d -> p (r d)", p=P)
                nc.sync.dma_start(dst, zero_big[:, : zn * D // P])
                zo += zn

            # --- aggregate: gather msg rows, combine dup-dst rows, scatter ---
            secs = [(0, APAD)] + ([(APAD, BPAD)] if BPAD else [])
            for b in range(C):
                for sec_o, sec_n in secs:
                    for o, n in _edge_chunks(sec_n):
                        oo = sec_o + o
                        gi = sb_idx.tile([P, CHUNK // 16], I16, tag="gi")
                        nc.sync.dma_start(
                            gi[:, : n // 16],
                            g_idx_t[b][:, oo // 16 : (oo + n) // 16],
                        )
                        si = sb_idx.tile([P, CHUNK // 16], I16, tag="si")
                        nc.sync.dma_start(
                            si[:, : n // 16],
                            s_idx_t[b][:, oo // 16 : (oo + n) // 16],
                        )
                        mt = sb_msg.tile([P, (CHUNK // P) * D], F32, tag="mt")
                        mtv = mt[:, : (n // P) * D].rearrange(
                            "p (q d) -> p q d", d=D
                        )
                        nc.gpsimd.dma_gather(
                            mtv, a2a_out[b], gi[:, : n // 16], n, n, D
                        )
                        if sec_o:
                            # section B: fold each dst run into its leader row
                            # via a selection matmul; non-leaders end up zero
                            # and their scatter idx points at the dump rows.
                            for gcol in range(n // P):
                                g = (oo - APAD) // P + gcol
                                ac = sb_idx.tile([P, 1], F32, tag="ac")
                                nc.sync.dma_start(
                                    ac[:], cmb_a_t[b][:, g : g + 1]
                                )
                                bc = sb_idx.tile([P, 1], F32, tag="bc")
                                nc.sync.dma_start(
                                    bc[:], cmb_b_t[b][:, g : g + 1]
                                )
                                pt = ps_tr.tile([P, P], F32, tag="pt")
                                nc.tensor.transpose(
                                    pt[:], bc[:].to_broadcast([P, P]), ident[:]
                                )
                                bT = sb_tr.tile([P, P], F32, tag="hT")
                                nc.vector.tensor_copy(bT[:], pt[:])
                                S = sb_tr.tile([P, P], F32, tag="aT")
                                nc.vector.tensor_tensor(
                                    S[:],
                                    ac[:].to_broadcast([P, P]),
                                    bT[:],
                                    op=mybir.AluOpType.is_equal,
                                )
                                mcol = mt[:, gcol * D : (gcol + 1) * D]
                                pm = ps_o.tile([P, D], F32, tag="po")
                                nc.tensor.matmul(
                                    pm[:], S[:], mcol, start=True, stop=True
                                )
                                nc.vector.tensor_copy(mcol, pm[:])
                        nc.gpsimd.dma_scatter_add(
                            agg[:, :], mtv, si[:, : n // 16], n, n, D
                        )

            # --- node update ---
            def layer_pairs_factory(h_cur=h_cur, layer=layer):
                cache = {}

                def get(s, q, t):
                    if s not in cache:
                        r0 = s * SLAB * P
                        rows = SLAB * P
                        hsl = sb_slab.tile([P, SLAB * D], F32, tag="h_slab")
                        nc.sync.dma_start(
                            hsl[:].rearrange("p (q d) -> p q d", d=D),
                            h_cur[r0 : r0 + rows, :].rearrange(
                                "(q p) d -> p q d", p=P
                            ),
                        )
                        asl = sb_slab.tile([P, SLAB * D], F32, tag="a_slab")
                        nc.sync.dma_start(
                            asl[:].rearrange("p (q d) -> p q d", d=D),
                            agg[r0 : r0 + rows, :].rearrange(
                                "(q p) d -> p q d", p=P
                            ),
                        )
                        cache.clear()
                        cache[s] = (hsl, asl)
                    hsl, asl = cache[s]
                    # scale agg tile by 1/deg (per-partition scalar)
                    nc.vector.tensor_scalar_mul(
                        asl[:, q * D : (q + 1) * D],
                        asl[:, q * D : (q + 1) * D],
                        invd_sb[:, t : t + 1],
                    )
                    pt_h = ps_tr.tile([P, P], F32, tag="pt")
                    nc.tensor.transpose(
                        pt_h[:], hsl[:, q * D : (q + 1) * D], ident[:]
                    )
                    hT = sb_tr.tile([P, P], F32, tag="hT")
                    nc.vector.tensor_copy(hT[:], pt_h[:])
                    pt_a = ps_tr.tile([P, P], F32, tag="pt")
                    nc.tensor.transpose(
                        pt_a[:], asl[:, q * D : (q + 1) * D], ident[:]
                    )
                    aT = sb_tr.tile([P, P], F32, tag="aT")
                    nc.vector.tensor_copy(aT[:], pt_a[:])
                    return [
                        (hT[:], wself_sb[layer][:]),
                        (aT[:], wneigh_sb[layer][:]),
                    ]

                return get

            act = AF.Copy if last else AF.Relu
            node_update(
                layer_pairs_factory(), b_sb[layer + 1][:], act, h_nxt, True,
            )

    nc.compile()
    return nc


def kernel(**inputs):
    C = 8
    meta, in_maps = preprocess(
        inputs["x"],
        inputs["edge_index"],
        inputs["W_in"],
        inputs["b_in"],
        inputs["W_self"],
        inputs["W_neigh"],
        inputs["b_layers"],
        C,
    )
    nc = build_nc(meta)
    res = run_bass_kernel_spmd(nc, in_maps, core_ids=list(range(C)))
    NSH = meta["NSH"]
    out = np.concatenate([r["out"][:NSH] for r in res.results], axis=0)
    return out.astype(np.float32)

